# revision 10
# baseline (speedup 1.0000x reference)
"""GNN message passing (SpMM) on 8 Trainium2 NeuronCores.

Computes out = segment_sum((X @ W)[cols] * vals, rows) for
X [50000, 128] f32, W [128, 128], 800k edges -- as
out = segment_sum(vals * X[cols]) @ W  (linearity): gather raw X rows
(bf16), accumulate per-destination via one-hot matmul-scatter on the
TensorEngine, apply W once per 128-destination block.

Sharding: destinations split across 8 cores (6250 each); edges
partitioned by destination.  Edges are packed CONTIGUOUSLY per
(chunk of 4 dest-blocks, source-window) -- no per-block 128-alignment
-- so gather descriptor padding is only per-chunk.  Blocks are
rank-matched across cores (sorted by edge count, chunk c takes ranks
c, c+13, c+26, c+39) so the static per-call gather sizes (max across
cores) stay tight.  Within a chunk each dest-block owns a static tile
range (union across cores); edges of neighboring blocks inside a
straddling tile are masked via val=0 in the one-hot.  The final W
matmul is emitted weight-stationary (lhsT=W) so the output leaves in
feature-major bf16, making the output DMA contiguous per partition.
"""

import numpy as np
import ml_dtypes

import concourse.bacc as bacc
import concourse.bass as bass
import concourse.mybir as mybir
import concourse.tile as tile
from concourse.bass_utils import run_bass_kernel_spmd

N_NODES = 50000
N_EDGES = 800000
F = 128
NCORES = 8
NPC = N_NODES // NCORES          # 6250 destinations per core
BLK = 128
NB = (NPC + BLK - 1) // BLK      # 49 blocks (last has 106 rows)
NWIN = 2
WINBASE = N_NODES // NWIN        # 25000 (< int16 max 32767)
# chunk sizes: mostly 4 blocks, but tiny final chunks so the compute
# tail after the last gather is short
CSIZES = [4] * 11 + [3, 1, 1]
assert sum(CSIZES) == NB
NCH = len(CSIZES)
CB = max(CSIZES)

BF16 = mybir.dt.bfloat16
F32 = mybir.dt.float32
I16 = mybir.dt.int16

_CACHE = {}


def _prep(adj_rows, adj_cols, adj_vals):
    """Host-side sharding. Returns (struct, idx, dmat, vmat, perm)."""
    dst = np.asarray(adj_rows).astype(np.int64)
    src = np.asarray(adj_cols).astype(np.int64)
    val = np.asarray(adj_vals).astype(np.float32)

    core = dst // NPC
    blk = (dst % NPC) // BLK
    drel = (dst % NPC) % BLK
    win = (src >= WINBASE).astype(np.int64)

    cnt = np.bincount(
        (core * NB + blk) * NWIN + win, minlength=NCORES * NB * NWIN
    ).reshape(NCORES, NB, NWIN)

    # rank blocks per core by total edge count; chunk c = ranks c+i*NCH
    perm = np.argsort(-cnt.sum(axis=2), axis=1, kind="stable")  # [NC, NB]
    rank_of_block = np.empty_like(perm)
    for k in range(NCORES):
        rank_of_block[k, perm[k]] = np.arange(NB)

    # deal ranks column-major over (slot position, chunk) so each chunk
    # mixes large and small blocks
    nslots = np.array(CSIZES)
    rk2c = np.empty(NB, dtype=np.int64)
    rk2i = np.empty(NB, dtype=np.int64)
    rank_of_ci = {}
    r = 0
    for i in range(CB):
        for c in range(NCH):
            if i < CSIZES[c]:
                rk2c[r], rk2i[r] = c, i
                rank_of_ci[(c, i)] = r
                r += 1
    assert r == NB
    rank_e = rank_of_block[core, blk]
    chunk_e = rk2c[rank_e]
    slot_e = rk2i[rank_e]

    # per-(core, chunk, win, slot) counts and stream offsets
    cnt4 = np.zeros((NCORES, NCH, NWIN, CB), dtype=np.int64)
    np.add.at(cnt4, (core, chunk_e, win, slot_e), 1)
    o4 = np.cumsum(cnt4, axis=3) - cnt4            # exclusive prefix
    S = cnt4.sum(axis=3)                           # [NC, NCH, NWIN]
    ncall = ((S.max(axis=0) + BLK - 1) // BLK) * BLK  # [NCH, NWIN]

    # static per-slot tile ranges (union across cores)
    a4 = o4 // BLK                                  # [NC, NCH, NWIN, CB]
    b4 = np.where(cnt4 > 0, (o4 + cnt4 + BLK - 1) // BLK, a4)
    a4m = np.where(cnt4 > 0, a4, np.iinfo(np.int64).max)
    A4 = a4m.min(axis=0)                            # [NCH, NWIN, CB]
    B4 = b4.max(axis=0)
    A4 = np.minimum(A4, B4)                         # empty -> A=B

    # dv column bases + static schedule
    dvbase = np.zeros((NCH, NWIN, CB), dtype=np.int64)
    sched = []
    sid_rank = []
    slot_start = []
    col = 0
    sid = 0
    for c in range(NCH):
        slot_start.append(sid)
        entries = []
        for i in range(int(nslots[c])):
            parts = []
            for w in range(NWIN):
                ntl = int(B4[c, w, i] - A4[c, w, i])
                if ntl > 0:
                    dvbase[c, w, i] = col
                    parts.append((w, int(A4[c, w, i]), ntl, col))
                    col += ntl
            assert parts, f"slot ({c},{i}) has no tiles"
            entries.append((sid, tuple(parts)))
            sid_rank.append(rank_of_ci[(c, i)])
            sid += 1
        sched.append(tuple(entries))
    ndv = col
    assert sid == NB

    idx_off = np.zeros((NCH, NWIN), dtype=np.int64)
    o = 0
    for c in range(NCH):
        for w in range(NWIN):
            idx_off[c, w] = o
            o += int(ncall[c, w]) // 16
    idx_cols = int(o)

    # per-edge placement
    key = ((core * NCH + chunk_e) * NWIN + win) * CB + slot_e
    order = np.lexsort((src, key))
    key_s = key[order]
    gstart = np.zeros(NCORES * NCH * NWIN * CB + 1, dtype=np.int64)
    np.cumsum(cnt4.ravel(), out=gstart[1:])
    rank_in = np.arange(N_EDGES, dtype=np.int64) - gstart[key_s]

    core_s = core[order]
    chunk_s = chunk_e[order]
    win_s = win[order]
    slot_s = slot_e[order]
    q = o4[core_s, chunk_s, win_s, slot_s] + rank_in
    lane = q % BLK
    tau = q // BLK
    dvcol = dvbase[chunk_s, win_s, slot_s] + (tau - A4[chunk_s, win_s, slot_s])

    dmat = np.zeros((NCORES, BLK, ndv), dtype=np.float32)
    dmat[core_s, lane, dvcol] = drel[order].astype(np.float32)
    vmat = np.zeros((NCORES, BLK, ndv), dtype=np.float32)
    vmat[core_s, lane, dvcol] = val[order].astype(np.float32)

    idxbase = np.zeros((NCORES, 16, idx_cols), dtype=np.int16)
    icol = idx_off[chunk_s, win_s] + q // 16
    irow = q % 16
    idxbase[core_s, irow, icol] = (src[order] - win_s * WINBASE).astype(np.int16)
    idx = np.tile(idxbase, (1, 8, 1))

    struct = dict(
        ncall=ncall, idx_off=idx_off, idx_cols=idx_cols, ndv=ndv,
        sched=tuple(sched), sid_rank=sid_rank, slot_start=slot_start,
        nslots=nslots,
    )
    return struct, idx, dmat, vmat, perm


def _build(struct, rep=1, gbufs=4):
    ncall = struct["ncall"]
    idx_off = struct["idx_off"]
    sched = struct["sched"]
    slot_start = struct["slot_start"]
    nslots = struct["nslots"]
    ndv = struct["ndv"]

    nc = bacc.Bacc("TRN2", debug=False)
    x = nc.declare_dram_parameter("x", [N_NODES, F], BF16, isOutput=False)
    wm = nc.declare_dram_parameter("wm", [F, F], BF16, isOutput=False)
    iotam = nc.declare_dram_parameter("iotam", [BLK, BLK], BF16, isOutput=False)
    idxp = nc.declare_dram_parameter(
        "idx", [BLK, struct["idx_cols"]], I16, isOutput=False
    )
    dmatp = nc.declare_dram_parameter("dmat", [BLK, ndv], F32, isOutput=False)
    vmatp = nc.declare_dram_parameter("vmat", [BLK, ndv], F32, isOutput=False)
    outp = nc.declare_dram_parameter("out", [F, NB * BLK], BF16, isOutput=True)

    xw = [x[0:WINBASE, :], x[WINBASE:N_NODES, :]]

    with tile.TileContext(nc) as tc:
        with (
            tc.tile_pool(name="const", bufs=1) as constp,
            tc.tile_pool(name="g0", bufs=gbufs) as g0p,
            tc.tile_pool(name="g1", bufs=gbufs) as g1p,
            tc.tile_pool(name="st", bufs=10) as stp,
            tc.tile_pool(name="psa", bufs=4, space="PSUM") as psap,
            tc.tile_pool(name="pso", bufs=3, space="PSUM") as psop,
            tc.tile_pool(name="acct", bufs=4) as acctp,
            tc.tile_pool(name="outs", bufs=4) as outsp,
        ):
            # compute-critical consts FIRST so they aren't queued behind
            # gathers on the DMA engines
            d_t = constp.tile([BLK, ndv], F32, tag="d_t")
            nc.sync.dma_start(out=d_t[:], in_=dmatp[:])
            v_t = constp.tile([BLK, ndv], F32, tag="v_t")
            nc.sync.dma_start(out=v_t[:], in_=vmatp[:])
            w_t = constp.tile([F, F], BF16, tag="w_t")
            nc.sync.dma_start(out=w_t[:], in_=wm[:])
            iota_t = constp.tile([BLK, BLK], BF16, tag="iota_t")
            nc.sync.dma_start(out=iota_t[:], in_=iotam[:])
            idx_tiles = {}
            for c in range(NCH):
                for w in range(NWIN):
                    n = int(ncall[c, w])
                    if n == 0:
                        continue
                    io = int(idx_off[c, w])
                    it = constp.tile([BLK, n // 16], I16, tag=f"idx_{c}_{w}")
                    nc.sync.dma_start(out=it[:], in_=idxp[:, io : io + n // 16])
                    idx_tiles[(c, w)] = it

            import contextlib

            loop_ctx = (
                tc.For_i(0, rep, 1) if rep > 1 else contextlib.nullcontext()
            )
            def compute_chunk(c, g):
                    ns_c = int(nslots[c])
                    lo = int(slot_start[c])
                    out_stage = outsp.tile([F, ns_c, BLK], BF16, tag="outs")
                    for sid, parts in sched[c]:
                        ntile_b = sum(p[2] for p in parts)
                        acc = psap.tile([F, BLK], F32, tag="acc")
                        k = 0
                        for w, a0, ntl, dvb in parts:
                            for j in range(ntl):
                                st = stp.tile([BLK, BLK], BF16, tag="st")
                                col = dvb + j
                                nc.vector.tensor_scalar(
                                    out=st[:],
                                    in0=iota_t[:],
                                    scalar1=d_t[:, col : col + 1],
                                    scalar2=v_t[:, col : col + 1],
                                    op0=mybir.AluOpType.is_equal,
                                    op1=mybir.AluOpType.mult,
                                )
                                nc.tensor.matmul(
                                    out=acc[:],
                                    lhsT=g[w][:, a0 + j, :],
                                    rhs=st[:],
                                    start=(k == 0),
                                    stop=(k == ntile_b - 1),
                                )
                                k += 1
                        acct = acctp.tile([F, BLK], BF16, tag="acct")
                        nc.scalar.copy(out=acct[:], in_=acc[:])
                        ops = psop.tile([F, BLK], F32, tag="ops")
                        # weight-stationary: out[f_out, d] = sum_k W[k,f] acc[k,d]
                        nc.tensor.matmul(
                            out=ops[:], lhsT=w_t[:], rhs=acct[:], start=True,
                            stop=True,
                        )
                        nc.scalar.copy(out=out_stage[:, sid - lo, :], in_=ops[:])
                    out_ap = outp[:, lo * BLK : (lo + ns_c) * BLK].rearrange(
                        "f (b d) -> f b d", d=BLK
                    )
                    nc.sync.dma_start(out=out_ap, in_=out_stage[:])

            with loop_ctx:
                for c in range(NCH):
                    g = [None, None]
                    for w in range(NWIN):
                        n = int(ncall[c, w])
                        if n == 0:
                            continue
                        gt = (g0p if w == 0 else g1p).tile(
                            [BLK, n // BLK, F], BF16, tag=f"g{w}"
                        )
                        nc.gpsimd.dma_gather(
                            gt[:], xw[w], idx_tiles[(c, w)][:], n, n, F,
                            single_packet=False,
                        )
                        g[w] = gt
                    compute_chunk(c, g)
    nc.compile()
    return nc


def kernel(input, weight, adj_rows, adj_cols, adj_vals):
    x = np.asarray(input, dtype=np.float32)
    w = np.asarray(weight, dtype=np.float32)

    struct, idx, dmat, vmat, perm = _prep(adj_rows, adj_cols, adj_vals)

    ckey = (struct["idx_cols"], struct["ndv"], struct["ncall"].tobytes(),
            struct["sched"])
    if ckey in _CACHE:
        nc = _CACHE[ckey]
    else:
        nc = _build(struct)
        _CACHE[ckey] = nc

    xb = x.astype(ml_dtypes.bfloat16)
    wb = w.astype(ml_dtypes.bfloat16)
    iota = np.tile(np.arange(BLK, dtype=np.float32), (BLK, 1)).astype(
        ml_dtypes.bfloat16
    )

    in_maps = [
        {"x": xb, "wm": wb, "iotam": iota, "idx": idx[k], "dmat": dmat[k],
         "vmat": vmat[k]}
        for k in range(NCORES)
    ]
    res = run_bass_kernel_spmd(nc, in_maps, core_ids=list(range(NCORES)))

    out = np.empty((N_NODES, F), dtype=np.float32)
    sid_rank = struct["sid_rank"]
    for k in range(NCORES):
        r = np.asarray(res.results[k]["out"]).astype(np.float32)  # [F, NB*BLK]
        rT = np.ascontiguousarray(r.T)                            # [NB*BLK, F]
        for s in range(NB):
            b = perm[k, sid_rank[s]]
            n = min(BLK, NPC - b * BLK)
            out[k * NPC + b * BLK : k * NPC + b * BLK + n] = rT[
                s * BLK : s * BLK + n
            ]
    return out
